# revision 1
# baseline (speedup 1.0000x reference)
"""Two-layer GAT (PyG GATConv semantics) as a Bass/Tile kernel on 8 TRN2 NeuronCores.

Strategy (graph/data parallel, dst-sharded):
  - Nodes padded to NPAD=50176, split into 8 contiguous shards of SHARD=6272
    (= 49 blocks x 128). Core k owns dst nodes [k*SHARD, (k+1)*SHARD).
  - Edges (incl. self loops) bucketed by dst shard, sorted by dst, split into
    two streams by src half (gather indices are int16, so gather tables are
    addressed as two halves of < 32768 rows each).
  - P1: every core redundantly computes the full projection table
    tab1[n] = [h(n) bf16(384) | a_src(n) f32(6) | a_dst(n) f32(6) | pad]
    (1024B rows), h = x@W1, a_* = h . att_* via folded weight columns.
  - P2: per dst block (128 nodes): dma_gather of tab1 rows for the block's
    in-edges; one-hot matrices U[e,d] / UT[d,e] built with is_equal compares;
    per-edge logits via a tiny PE matmul (UT.T @ a_dst_blk) + gathered a_src;
    leaky-relu + Exp; aggregation [num | den] = U.T @ [ex*h | ex] accumulated
    in PSUM over the block's chunks; then out = num/den (+bias, ReLU) -> h1.
  - P3: h2e = h1 @ [W2 | W2@att_src2 | W2@att_dst2] for own shard; AllGather
    into the replicated layer-2 table tab2 (256B rows).
  - P4: same edge machinery for layer 2 (1 head, 32 ch) -> y (own shard).

kernel() takes full inputs, preprocesses indices on the host (sorting /
bucketing / layout only), compiles one SPMD NEFF, runs it on cores 0..7 via
bass_utils.run_bass_kernel_spmd, and concatenates the per-core outputs.
"""

import contextlib
import dataclasses

import numpy as np

import concourse.bass as bass
import concourse.mybir as mybir
import concourse.tile as tile
import concourse.bacc as bacc
from concourse.bass_utils import run_bass_kernel_spmd
from concourse.alu_op_type import AluOpType

F32 = mybir.dt.float32
BF16 = mybir.dt.bfloat16
I16 = mybir.dt.int16

PAD_OFF = 200.0  # dst_off sentinel for padding edges -> one-hot column all-zero


@dataclasses.dataclass
class Cfg:
    N: int = 50000
    E: int = 800000
    IN: int = 256
    HEADS: int = 6
    HID: int = 64
    OUT: int = 32
    NEG: float = 0.2
    NC: int = 8
    NB: int = 49
    BLK: int = 128
    skip_cc: bool = False   # debug: replace AllGather with local copy (wrong results)

    @property
    def D1(self):
        return self.HEADS * self.HID

    @property
    def SHARD(self):
        return self.NB * self.BLK

    @property
    def NPAD(self):
        return self.NC * self.SHARD

    @property
    def HALF(self):
        return self.NPAD // 2

    @property
    def ROW1(self):
        need = self.D1 // 2 + 2 * self.HEADS
        return ((need + 63) // 64) * 64          # f32 elems / tab1 row

    @property
    def ROW2(self):
        need = self.OUT + 4
        return ((need + 127) // 128) * 128       # bf16 elems / tab2 row


def _wrap_idx(idx_flat):
    """int16 gather index layout: index i at [partition i%16, free i//16],
    replicated down to 128 partitions."""
    n = idx_flat.shape[0]
    assert n % 16 == 0
    w = idx_flat.reshape(-1, 16).T.astype(np.int16)
    return np.tile(w, (8, 1))


@dataclasses.dataclass
class EdgePlan:
    chA: list
    chB: list
    nreal: list
    G1: int
    idx: np.ndarray
    dstoff_col: np.ndarray
    dstoff_row: np.ndarray
    totw: int


def build_edge_plan(cfg: Cfg, edge_index: np.ndarray) -> EdgePlan:
    N, NC, NB, BLK = cfg.N, cfg.NC, cfg.NB, cfg.BLK
    SHARD, HALF = cfg.SHARD, cfg.HALF
    src = np.concatenate([np.asarray(edge_index[0], np.int64),
                          np.arange(N, dtype=np.int64)])
    dst = np.concatenate([np.asarray(edge_index[1], np.int64),
                          np.arange(N, dtype=np.int64)])
    core = dst // SHARD
    blk = (dst % SHARD) // BLK
    off = (dst % BLK).astype(np.float32)
    strm = (src >= HALF).astype(np.int64)
    lsrc = (src - strm * HALF).astype(np.int32)

    slot = (core * NB + blk) * 2 + strm
    order = np.argsort(slot, kind="stable")
    slot_s, lsrc_s, off_s = slot[order], lsrc[order], off[order]
    counts = np.bincount(slot_s, minlength=NC * NB * 2)
    starts = np.concatenate([[0], np.cumsum(counts)])

    cnt = counts.reshape(NC, NB, 2)
    ch = np.maximum(1, -(-cnt.max(axis=0) // BLK))
    chA, chB = ch[:, 0].tolist(), ch[:, 1].tolist()
    nreal = [a + b for a, b in zip(chA, chB)]
    G1 = int(sum(nreal))
    RW = -(-G1 // 128) * 128
    totw = sum((1 + a + 1 + b) * (BLK // 16) for a, b in zip(chA, chB))

    idx_all = np.zeros((NC, 128, totw), np.int16)
    dcol = np.full((NC, 128, G1), PAD_OFF, np.float32)
    drow = np.full((NC, 128, RW), PAD_OFF, np.float32)

    for k in range(NC):
        own_half = 0 if (k * SHARD) < HALF else 1
        wpos = 0
        g = 0
        for b in range(NB):
            for s, nch in ((0, chA[b]), (1, chB[b])):
                seg = np.zeros(((1 + nch) * BLK,), np.int32)
                if s == own_half:
                    own0 = k * SHARD + b * BLK - s * HALF
                    seg[:BLK] = own0 + np.arange(BLK)
                sidx = (k * NB + b) * 2 + s
                st, en = starts[sidx], starts[sidx + 1]
                cntk = en - st
                assert cntk <= nch * BLK
                seg[BLK:BLK + cntk] = lsrc_s[st:en]
                w = _wrap_idx(seg)
                idx_all[k][:, wpos:wpos + w.shape[1]] = w
                wpos += w.shape[1]
                offs = np.full((nch * BLK,), PAD_OFF, np.float32)
                offs[:cntk] = off_s[st:en]
                offs = offs.reshape(nch, BLK)
                for c in range(nch):
                    dcol[k][:, g] = offs[c]
                    drow[k][g % 128, (g // 128) * 128:(g // 128 + 1) * 128] = offs[c]
                    g += 1
        assert g == G1 and wpos == totw
    return EdgePlan(chA, chB, nreal, G1, idx_all, dcol, drow, totw)


def build_nc(cfg: Cfg, plan: EdgePlan):
    c = cfg
    nc = bacc.Bacc("TRN2", target_bir_lowering=False, debug=False,
                   enable_asserts=False, num_devices=c.NC,
                   num_swdge_queues=4)

    H = c.HEADS
    D1, IN, OUT = c.D1, c.IN, c.OUT
    NBK = c.NPAD // 128
    KIN = IN // 128
    K1 = D1 // 128

    xt = nc.dram_tensor("xt", [IN, c.NPAD], BF16, kind="ExternalInput")
    w1 = nc.dram_tensor("w1", [IN, D1], BF16, kind="ExternalInput")
    w1t = nc.dram_tensor("w1t", [D1, IN], BF16, kind="ExternalInput")
    attbd1 = nc.dram_tensor("attbd1", [D1, 2 * H], BF16, kind="ExternalInput")
    w2 = nc.dram_tensor("w2", [D1, OUT], BF16, kind="ExternalInput")
    w2t = nc.dram_tensor("w2t", [OUT, D1], BF16, kind="ExternalInput")
    att2 = nc.dram_tensor("att2", [OUT, 2], BF16, kind="ExternalInput")
    b1r = nc.dram_tensor("b1r", [128, D1], F32, kind="ExternalInput")
    b2r = nc.dram_tensor("b2r", [128, OUT], F32, kind="ExternalInput")
    iota_r = nc.dram_tensor("iota_r", [128, 128], BF16, kind="ExternalInput")
    iota_c = nc.dram_tensor("iota_c", [128, 1], F32, kind="ExternalInput")
    ident = nc.dram_tensor("ident", [128, 128], BF16, kind="ExternalInput")
    ind_a = nc.dram_tensor("ind_a", [128, 1], F32, kind="ExternalInput")
    ind_b = nc.dram_tensor("ind_b", [128, 1], F32, kind="ExternalInput")
    idx_d = nc.dram_tensor("idx_d", [128, plan.totw], I16, kind="ExternalInput")
    dcol_d = nc.dram_tensor("dcol_d", [128, plan.G1], F32, kind="ExternalInput")

    tab1 = nc.dram_tensor("tab1", [c.NPAD, c.ROW1], F32)
    h2e_own = nc.dram_tensor("h2e_own", [c.SHARD, c.ROW2], BF16)
    tab2 = nc.dram_tensor("tab2", [c.NPAD, c.ROW2], BF16, addr_space="Shared")
    y = nc.dram_tensor("y", [c.SHARD, OUT], F32, kind="ExternalOutput")

    AS0 = D1 // 2            # f32 col of a_src in a tab1 row
    AD0 = AS0 + H            # f32 col of a_dst in a tab1 row

    with tile.TileContext(nc, num_cores=c.NC) as tc:
        with contextlib.ExitStack() as ctx:
            consts = ctx.enter_context(tc.tile_pool(name="consts", bufs=1))
            h1tp = ctx.enter_context(tc.tile_pool(name="h1t", bufs=1))
            projx = ctx.enter_context(tc.tile_pool(name="projx", bufs=4))
            stg = ctx.enter_context(tc.tile_pool(name="stg", bufs=4))
            gp = ctx.enter_context(tc.tile_pool(name="gath", bufs=3))
            up = ctx.enter_context(tc.tile_pool(name="upool", bufs=2))
            wk = ctx.enter_context(tc.tile_pool(name="wk", bufs=6))
            idxp = ctx.enter_context(tc.tile_pool(name="idxp", bufs=4))
            pmm = ctx.enter_context(tc.tile_pool(name="pmm", bufs=2, space="PSUM"))
            ppa = ctx.enter_context(tc.tile_pool(name="ppa", bufs=2, space="PSUM"))
            paux = ctx.enter_context(tc.tile_pool(name="paux", bufs=4, space="PSUM"))

            def load_const(dram, shape, dtype):
                t = consts.tile(shape, dtype, tag=dram.name)
                nc.sync.dma_start(t[:], dram.ap())
                return t

            iota_row = load_const(iota_r, [128, 128], BF16)
            iota_col = load_const(iota_c, [128, 1], F32)
            ident_b = load_const(ident, [128, 128], BF16)
            b1_sb = load_const(b1r, [128, D1], F32)
            b2_sb = load_const(b2r, [128, OUT], F32)
            indA = load_const(ind_a, [128, 1], F32)
            indB = load_const(ind_b, [128, 1], F32)
            dcol_sb = load_const(dcol_d, [128, plan.G1], F32)

            # ---- W1e [128, KIN, D1+2H] and W2e [128, K1, OUT+2] ----
            w1e = consts.tile([128, KIN, D1 + 2 * H], BF16, tag="w1e")
            for ki in range(KIN):
                nc.sync.dma_start(w1e[:, ki, 0:D1],
                                  w1.ap()[ki * 128:(ki + 1) * 128, :])
            w1t_s = consts.tile([128, K1, IN], BF16, tag="w1t_s")
            for kj in range(K1):
                nc.sync.dma_start(w1t_s[:, kj, :],
                                  w1t.ap()[kj * 128:(kj + 1) * 128, :])
            abd_s = consts.tile([128, K1, 2 * H], BF16, tag="abd_s")
            for kj in range(K1):
                nc.sync.dma_start(abd_s[:, kj, :],
                                  attbd1.ap()[kj * 128:(kj + 1) * 128, :])
            for ki in range(KIN):
                ps = paux.tile([128, 2 * H], F32, tag="aux")
                for kj in range(K1):
                    nc.tensor.matmul(ps[:], w1t_s[:, kj, ki * 128:(ki + 1) * 128],
                                     abd_s[:, kj, :], start=(kj == 0),
                                     stop=(kj == K1 - 1))
                nc.scalar.copy(w1e[:, ki, D1:D1 + 2 * H], ps[:])

            w2e = consts.tile([128, K1, OUT + 2], BF16, tag="w2e")
            for kj in range(K1):
                nc.sync.dma_start(w2e[:, kj, 0:OUT],
                                  w2.ap()[kj * 128:(kj + 1) * 128, :])
            w2t_s = consts.tile([128, D1], BF16, tag="w2t_s")
            nc.sync.dma_start(w2t_s[:OUT, :], w2t.ap())
            att2_s = consts.tile([128, 2], BF16, tag="att2_s")
            nc.sync.dma_start(att2_s[:OUT, :], att2.ap())
            for kj in range(K1):
                ps = paux.tile([128, 2], F32, tag="aux")
                nc.tensor.matmul(ps[:], w2t_s[:OUT, kj * 128:(kj + 1) * 128],
                                 att2_s[:OUT, :], start=True, stop=True)
                nc.scalar.copy(w2e[:, kj, OUT:OUT + 2], ps[:])

            # ---- P1: replicated projection -> tab1 ----
            assert NBK % 2 == 0
            for nb2 in range(NBK // 2):
                xts = []
                for ki in range(KIN):
                    xtile = projx.tile([128, 256], BF16, tag="xt")
                    nc.sync.dma_start(
                        xtile[:], xt.ap()[ki * 128:(ki + 1) * 128,
                                          nb2 * 256:(nb2 + 1) * 256])
                    xts.append(xtile)
                for half in range(2):
                    nb = nb2 * 2 + half
                    ps = pmm.tile([128, D1 + 2 * H], F32, tag="mm")
                    for ki in range(KIN):
                        nc.tensor.matmul(
                            ps[:], xts[ki][:, half * 128:(half + 1) * 128],
                            w1e[:, ki, :], start=(ki == 0),
                            stop=(ki == KIN - 1))
                    st = stg.tile([128, c.ROW1], F32, tag="stage1")
                    nc.vector.memset(st[:, AD0 + H:c.ROW1], 0.0)
                    nc.scalar.copy(st[:, 0:AS0].bitcast(BF16), ps[:, 0:D1])
                    nc.vector.tensor_copy(st[:, AS0:AD0 + H],
                                          ps[:, D1:D1 + 2 * H])
                    nc.sync.dma_start(tab1.ap()[nb * 128:(nb + 1) * 128, :],
                                      st[:])

            tc.strict_bb_all_engine_barrier()

            # ---- shared edge phase ----
            self_q = [0]

            def edge_phase(tabv_a, tabv_b, row_elems, row_dtype, nh, chans,
                           as_col, ad_col, out_cb, tag):
                wseg = 0
                g = 0
                for b in range(c.NB):
                    nA, nB_ = plan.chA[b], plan.chB[b]
                    nr = nA + nB_
                    gts = []
                    for s, nch in ((0, nA), (1, nB_)):
                        ni = (1 + nch) * 128
                        it = idxp.tile([128, ni // 16], I16, tag=f"idx{tag}")
                        nc.sync.dma_start(it[:],
                                          idx_d.ap()[:, wseg:wseg + ni // 16])
                        wseg += ni // 16
                        gt = gp.tile([128, 1 + nch, row_elems], row_dtype,
                                     tag=f"g{tag}{s}")
                        # ring holds 1024 descs; one gather emits num_idxs
                        # descs -> split into pieces of <= 7 chunks (896)
                        po = 0
                        while po < 1 + nch:
                            pc = min(7, 1 + nch - po)
                            nc.gpsimd.dma_gather(
                                gt[:, po:po + pc, :],
                                tabv_a if s == 0 else tabv_b,
                                it[:, po * 8:(po + pc) * 8],
                                pc * 128, pc * 128, row_elems,
                                queue_num=self_q[0])
                            self_q[0] = (self_q[0] + 1) % 4
                            po += pc
                        gts.append(gt)
                    gA, gB = gts

                    adst = wk.tile([128, nh], F32, tag=f"adst{tag}")
                    adst_b = wk.tile([128, nh], BF16, tag=f"adstb{tag}")
                    ga_a = gA[:, 0:1, :].bitcast(F32)[:, 0, ad_col:ad_col + nh]
                    ga_b = gB[:, 0:1, :].bitcast(F32)[:, 0, ad_col:ad_col + nh]
                    nc.vector.tensor_scalar(adst[:], ga_a, indA[:, 0:1], None,
                                            op0=AluOpType.mult)
                    tmpb = wk.tile([128, nh], F32, tag=f"adst2{tag}")
                    nc.vector.tensor_scalar(tmpb[:], ga_b, indB[:, 0:1], None,
                                            op0=AluOpType.mult)
                    nc.vector.tensor_tensor(adst[:], adst[:], tmpb[:],
                                            op=AluOpType.add)
                    nc.vector.tensor_copy(adst_b[:], adst[:])

                    paE = ppa.tile([128, nh * nr], F32, tag="pa")
                    uall = up.tile([128, nr * 128], BF16, tag=f"ua{tag}")
                    for r in range(nr):
                        gg = g + r
                        us = uall[:, r * 128:(r + 1) * 128]
                        nc.vector.tensor_scalar(
                            us, iota_row[:], dcol_sb[:, gg:gg + 1], None,
                            op0=AluOpType.is_equal)
                        pst = paux.tile([128, 128], BF16, tag="aux")
                        nc.tensor.transpose(pst[:], us, ident_b[:])
                        UT = wk.tile([128, 128], BF16, tag=f"UT{tag}")
                        nc.scalar.copy(UT[:], pst[:])
                        nc.tensor.matmul(paE[:, r * nh:(r + 1) * nh], UT[:],
                                         adst_b[:], start=True, stop=True)

                    esum = wk.tile([128, nh * nr], F32, tag=f"es{tag}")
                    for s, nch, base in ((0, nA, 0), (1, nB_, nA)):
                        if nch == 0:
                            continue
                        gt = gA if s == 0 else gB
                        asrc = gt[:, 1:1 + nch, :].bitcast(F32)[
                            :, :, as_col:as_col + nh]
                        pv = paE[:, base * nh:(base + nch) * nh].rearrange(
                            "p (ch h) -> p ch h", h=nh)
                        ev = esum[:, base * nh:(base + nch) * nh].rearrange(
                            "p (ch h) -> p ch h", h=nh)
                        nc.vector.tensor_tensor(ev, asrc, pv, op=AluOpType.add)
                    # leaky relu: max(x, neg*x), then exp
                    lk = wk.tile([128, nh * nr], F32, tag=f"lk{tag}")
                    nc.vector.tensor_scalar(lk[:], esum[:], c.NEG, None,
                                            op0=AluOpType.mult)
                    nc.vector.tensor_tensor(lk[:], lk[:], esum[:],
                                            op=AluOpType.max)
                    ex = wk.tile([128, nh * nr], BF16, tag=f"ex{tag}")
                    nc.scalar.activation(ex[:], lk[:],
                                         mybir.ActivationFunctionType.Exp)

                    pnum = pmm.tile([128, chans + nh], F32, tag="mm")
                    for r in range(nr):
                        s = 0 if r < nA else 1
                        cpos = 1 + (r if s == 0 else r - nA)
                        gt = gA if s == 0 else gB
                        U = uall[:, r * 128:(r + 1) * 128]
                        hw = wk.tile([128, chans + nh], BF16, tag=f"hw{tag}")
                        exs = ex[:, r * nh:(r + 1) * nh]
                        if row_dtype == F32:
                            hview = gt[:, cpos:cpos + 1, 0:chans // 2] \
                                .bitcast(BF16).rearrange(
                                    "p o (h ch) -> p (o h) ch", h=nh)
                        else:
                            hview = gt[:, cpos:cpos + 1, 0:chans].rearrange(
                                "p o (h ch) -> p (o h) ch", h=nh)
                        exb = exs.broadcast_to([128, nh, chans // nh])
                        hwv = hw[:, 0:chans].rearrange(
                            "p (h ch) -> p h ch", h=nh)
                        nc.vector.tensor_tensor(hwv, hview, exb,
                                                op=AluOpType.mult)
                        nc.vector.tensor_copy(hw[:, chans:chans + nh], exs)
                        nc.tensor.matmul(pnum[:], U, hw[:],
                                         start=(r == 0), stop=(r == nr - 1))
                    out_cb(b, pnum)
                    g += nr

            # ---- P2: layer-1 edges ----
            tabA1 = tab1.ap()[0:c.HALF, :]
            tabB1 = tab1.ap()[c.HALF:c.NPAD, :]
            h1T = h1tp.tile([128, K1, c.SHARD], BF16, tag="h1T")

            def l1_out(b, pnum):
                den = wk.tile([128, H], F32, tag="den1")
                nc.vector.tensor_scalar(den[:], pnum[:, D1:D1 + H], 1e-30, None,
                                        op0=AluOpType.max)
                rec = wk.tile([128, H], F32, tag="rec1")
                nc.vector.reciprocal(rec[:], den[:])
                tmp = wk.tile([128, D1], F32, tag="tmp1")
                nv = pnum[:, 0:D1].rearrange("p (h ch) -> p h ch", h=H)
                rb = rec[:].broadcast_to([128, H, c.HID])
                tv = tmp[:].rearrange("p (h ch) -> p h ch", h=H)
                nc.vector.tensor_tensor(tv, nv, rb, op=AluOpType.mult)
                nc.vector.tensor_tensor(tmp[:], tmp[:], b1_sb[:],
                                        op=AluOpType.add)
                h1s = wk.tile([128, D1], BF16, tag="h1s")
                nc.scalar.activation(h1s[:], tmp[:],
                                     mybir.ActivationFunctionType.Relu)
                for j in range(K1):
                    pst = paux.tile([128, 128], BF16, tag="aux")
                    nc.tensor.transpose(pst[:], h1s[:, j * 128:(j + 1) * 128],
                                        ident_b[:])
                    nc.scalar.copy(h1T[:, j, b * 128:(b + 1) * 128], pst[:])

            edge_phase(tabA1, tabB1, c.ROW1, F32, H, D1, AS0, AD0, l1_out, "1")

            # ---- P3: layer-2 table + AllGather ----
            for b in range(c.NB):
                ps = paux.tile([128, OUT + 2], F32, tag="aux")
                for kj in range(K1):
                    nc.tensor.matmul(ps[:], h1T[:, kj, b * 128:(b + 1) * 128],
                                     w2e[:, kj, :], start=(kj == 0),
                                     stop=(kj == K1 - 1))
                st2 = stg.tile([128, c.ROW2], BF16, tag="stage2")
                nc.vector.memset(st2[:, OUT + 4:c.ROW2], 0.0)
                nc.vector.tensor_copy(st2[:, 0:OUT], ps[:, 0:OUT])
                nc.vector.tensor_copy(st2[:, OUT:OUT + 4].bitcast(F32),
                                      ps[:, OUT:OUT + 2])
                nc.sync.dma_start(h2e_own.ap()[b * 128:(b + 1) * 128, :],
                                  st2[:])
            tc.strict_bb_all_engine_barrier()
            if c.skip_cc:
                for q in range(c.NC):
                    nc.sync.dma_start(
                        tab2.ap()[q * c.SHARD:(q + 1) * c.SHARD, :],
                        h2e_own.ap())
            else:
                nc.gpsimd.collective_compute(
                    "AllGather", AluOpType.bypass,
                    replica_groups=[list(range(c.NC))],
                    ins=[h2e_own.ap()], outs=[tab2.ap()])
            tc.strict_bb_all_engine_barrier()

            # ---- P4: layer-2 edges ----
            tabA2 = tab2.ap()[0:c.HALF, :]
            tabB2 = tab2.ap()[c.HALF:c.NPAD, :]
            AS2 = OUT // 2  # f32 col of a_src2 in a tab2 row

            def l2_out(b, pnum):
                den = wk.tile([128, 1], F32, tag="den2")
                nc.vector.tensor_scalar(den[:], pnum[:, OUT:OUT + 1], 1e-30,
                                        None, op0=AluOpType.max)
                rec = wk.tile([128, 1], F32, tag="rec2")
                nc.vector.reciprocal(rec[:], den[:])
                tmp = wk.tile([128, OUT], F32, tag="tmp2")
                nc.vector.tensor_scalar(tmp[:], pnum[:, 0:OUT], rec[:, 0:1],
                                        None, op0=AluOpType.mult)
                nc.vector.tensor_tensor(tmp[:], tmp[:], b2_sb[:],
                                        op=AluOpType.add)
                nc.sync.dma_start(y.ap()[b * 128:(b + 1) * 128, :], tmp[:])

            edge_phase(tabA2, tabB2, c.ROW2, BF16, 1, OUT, AS2, AS2 + 1,
                       l2_out, "2")

    nc.compile()
    return nc


def host_inputs(cfg: Cfg, plan: EdgePlan, x, W1, att_src1, att_dst1, b1, W2,
                att_src2, att_dst2, b2):
    c = cfg
    H = c.HEADS

    def bf(a):
        import ml_dtypes
        return np.asarray(a, np.float32).astype(ml_dtypes.bfloat16)

    xt = np.zeros((c.IN, c.NPAD), np.float32)
    xt[:, :c.N] = np.asarray(x, np.float32).T
    attbd1 = np.zeros((c.D1, 2 * H), np.float32)
    a_s1 = np.asarray(att_src1, np.float32).reshape(H, c.HID)
    a_d1 = np.asarray(att_dst1, np.float32).reshape(H, c.HID)
    for h in range(H):
        attbd1[h * c.HID:(h + 1) * c.HID, h] = a_s1[h]
        attbd1[h * c.HID:(h + 1) * c.HID, H + h] = a_d1[h]
    att2 = np.stack([np.asarray(att_src2, np.float32).reshape(c.OUT),
                     np.asarray(att_dst2, np.float32).reshape(c.OUT)], axis=1)

    base = {
        "xt": bf(xt),
        "w1": bf(W1),
        "w1t": bf(np.ascontiguousarray(np.asarray(W1, np.float32).T)),
        "attbd1": bf(attbd1),
        "w2": bf(W2),
        "w2t": bf(np.ascontiguousarray(np.asarray(W2, np.float32).T)),
        "att2": bf(att2),
        "b1r": np.tile(np.asarray(b1, np.float32).reshape(1, c.D1), (128, 1)),
        "b2r": np.tile(np.asarray(b2, np.float32).reshape(1, c.OUT), (128, 1)),
        "iota_r": bf(np.tile(np.arange(128, dtype=np.float32)[None, :],
                             (128, 1))),
        "iota_c": np.arange(128, dtype=np.float32)[:, None],
        "ident": bf(np.eye(128, dtype=np.float32)),
    }
    in_maps = []
    for k in range(c.NC):
        own_a = 1.0 if (k * c.SHARD) < c.HALF else 0.0
        m = dict(base)
        m["ind_a"] = np.full((128, 1), own_a, np.float32)
        m["ind_b"] = np.full((128, 1), 1.0 - own_a, np.float32)
        m["idx_d"] = plan.idx[k]
        m["dcol_d"] = plan.dstoff_col[k]
        in_maps.append(m)
    return in_maps


_CACHE = {}
LAST_RES = None


def kernel(x, edge_index, W1, att_src1, att_dst1, b1, W2, att_src2, att_dst2,
           b2, _cfg=None, _runner=None, _trace=False):
    cfg = _cfg or Cfg()
    ei = np.asarray(edge_index)
    plan = build_edge_plan(cfg, ei)
    key = (cfg.N, cfg.E, cfg.skip_cc, tuple(plan.chA), tuple(plan.chB))
    if key not in _CACHE:
        _CACHE[key] = build_nc(cfg, plan)
    nc = _CACHE[key]
    in_maps = host_inputs(cfg, plan, x, W1, att_src1, att_dst1, b1, W2,
                          att_src2, att_dst2, b2)
    global LAST_RES
    if _runner is not None:
        results = _runner(nc, in_maps)
    else:
        try:
            res = run_bass_kernel_spmd(nc, in_maps,
                                       core_ids=list(range(cfg.NC)),
                                       trace=_trace)
        except ModuleNotFoundError:
            if not _trace:
                raise
            # NTFF profiling hook unavailable in this environment
            res = run_bass_kernel_spmd(nc, in_maps,
                                       core_ids=list(range(cfg.NC)))
        LAST_RES = res
        results = res.results
    out = np.concatenate([results[k]["y"] for k in range(cfg.NC)], axis=0)
    return np.ascontiguousarray(out[:cfg.N]).astype(np.float32)

